# revision 44
# baseline (speedup 1.0000x reference)
"""Trainium2 Bass kernel for nn_NeuralODE: 196 Tsit5 steps of a 3->64->64->3
tanh MLP vector field over batch 32768, data-parallel across 8 NeuronCores.

Formulation:
  State is y itself ([3] per batch row), kept exactly in fp32 in a [6, 1024]
  tile per wave-pair (2 batch halves x 3 dims on partitions).  A packed fp32r
  "stack" tile [102, 1024] per pair holds an fp32r image of y (rows 0-5), a
  ones row (6), and kt_j at 32-aligned bases 32/64/96 for j=1..3 (engine
  writes must be 32-partition aligned), where kt_j := h2_j @ W3.  Per stage i
  the layer-1 preactivation
     pre_i = W1^T y + sum_{j<i} (h A_ij) W1^T k_j + const
  is ONE K=102 matmul over the stack (y and kt_{j<=i-2}) plus ONE
  accumulating K=128 matmul of h2_{i-1} against scaled Ghat = W3 @ W1 (stage
  6 adds one more for h2_4, whose kt has no aligned slot).  W2 is a
  block-diagonal K=128 matmul on [64 feats x 2 halves].  The step update
     y' = y + sum_j (h B_j) k_j
  is 4 accumulating matmuls into a [6, 1024] PSUM (stack kt_1..3 +
  W3-contractions of h2_4..6), an exact fp32 VectorE add into the y tile,
  and a VectorE copy refreshing the stack's fp32r y image.  Constants
  (b1/b2/b3 terms) fold into ACT bias columns / stationary rows (zero here).

  Compute granularity: 4 PE waves of 512 columns (25 matmul passes per step
  per wave), PAIRED into 2 streams for ScalarE/VectorE so each tanh covers
  [128, 1024] (halves the per-instruction overhead on the binding engine).
  kext matmuls+copies are emitted one stage late so they fill PE idle slots
  instead of blocking the next stage's preacts.  PSUM: 2x [128,1024] big
  tags + 2x [6,1024] small tags = exactly 8 banks.
"""
import numpy as np

import concourse.bacc as bacc
import concourse.bass as bass
import concourse.mybir as mybir
from concourse.bass import ds
from concourse.tile import TileContext
from concourse.bass_utils import run_bass_kernel_spmd

F32 = mybir.dt.float32
F32R = mybir.dt.float32r
TANH = mybir.ActivationFunctionType.Tanh

N_CORES = 8
T, B, D, W = 50, 32768, 3, 64
SUB = 4
N_INT = T - 1            # 49 save intervals
WAVES = 4                # PE streams of FREE columns each
PAIRS = WAVES // 2       # ACT/DVE streams of 2*FREE columns each
FREE = B // N_CORES // WAVES // 2   # 512
PFREE = 2 * FREE         # 1024: free dim of paired tiles
# NOTE: each matmul must write a full half of a paired PSUM tile (one bank);
# two accumulation groups must never share a PSUM bank (start=True clears
# has_written for the whole bank).
SROWS = 102              # stack rows: 6 y, 1 ones, kt_j at 32j..32j+5
ONES_ROW = 6

# Tsit5 tableau (matches reference.py)
_A = np.zeros((7, 7))
_A[2, 1] = 0.161
_A[3, 1], _A[3, 2] = -0.008480655492356989, 0.335480655492357
_A[4, 1], _A[4, 2], _A[4, 3] = 2.8971530571054935, -6.359448489975075, 4.3622954328695815
_A[5, 1], _A[5, 2], _A[5, 3], _A[5, 4] = (
    5.325864828439257, -11.748883564062828, 7.4955393428898365, -0.09249506636175525)
_A[6, 1], _A[6, 2], _A[6, 3], _A[6, 4], _A[6, 5] = (
    5.86145544294642, -12.92096931784711, 8.159367898576159,
    -0.071584973281401, -0.028269050394068383)
_B = np.array([0.0, 0.09646076681806523, 0.01, 0.4798896504144996,
               1.379008574103742, -3.290069515436081, 2.324710524099774])

LAST_EXEC_NS = None


def _krow(j: int) -> int:
    """First stack row of ktilde_j (j = 1..3) — 32-partition aligned."""
    return 32 * j


def _round_fp32r(x: np.ndarray) -> np.ndarray:
    """Round fp32 array to the fp32r grid (11-bit mantissa, RNE-ish)."""
    u = np.ascontiguousarray(np.asarray(x, dtype=np.float32)).view(np.uint32)
    r = (u + np.uint32(0x7FF) + ((u >> np.uint32(12)) & np.uint32(1))) & np.uint32(0xFFFFF000)
    return r.view(np.float32)


def _blk(m64: np.ndarray) -> np.ndarray:
    """Duplicate a [64,64] matrix into a block-diagonal [128,128]."""
    z = np.zeros((128, 128), dtype=np.float64)
    z[0:64, 0:64] = m64
    z[64:128, 64:128] = m64
    return z


def build(n_intervals: int = N_INT, body_reps: int = 1, loop_mult: int = 1,
          static_save: bool = False):
    nc = bacc.Bacc(None, target_bir_lowering=False)

    stk0_d = nc.dram_tensor("stk0", [PAIRS, SROWS, PFREE], F32R, kind="ExternalInput")
    y0p_d = nc.dram_tensor("y0p", [PAIRS, 6, PFREE], F32, kind="ExternalInput")
    sws_d = nc.dram_tensor("sws", [7, SROWS, 128], F32R, kind="ExternalInput")
    gbig_d = nc.dram_tensor("gbig", [10, 128, 128], F32R, kind="ExternalInput")
    w3s_d = nc.dram_tensor("w3s", [128, 24], F32R, kind="ExternalInput")
    u1_d = nc.dram_tensor("u1", [SROWS, 6], F32R, kind="ExternalInput")
    bia_d = nc.dram_tensor("biases", [128, 9], F32, kind="ExternalInput")
    ys_d = nc.dram_tensor("ys", [n_intervals, 6 * PAIRS, PFREE], F32,
                          kind="ExternalOutput")

    with TileContext(nc) as tc:
        with tc.tile_pool(name="wpool", bufs=1) as wpool, \
             tc.tile_pool(name="spool", bufs=1) as spool, \
             tc.tile_pool(name="h1pool", bufs=2) as h1pool, \
             tc.tile_pool(name="pbig", bufs=1, space="PSUM") as pbig, \
             tc.tile_pool(name="psmall", bufs=1, space="PSUM") as psmall:

            # --- weights / constants to SBUF ---
            sws = []
            for i in range(7):
                t = wpool.tile([SROWS, 128], F32R, name=f"sw{i}")
                nc.sync.dma_start(out=t[:, :], in_=sws_d[i, :, :])
                sws.append(t)
            gbig = []
            for i in range(10):
                t = wpool.tile([128, 128], F32R, name=f"gb{i}")
                nc.sync.dma_start(out=t[:, :], in_=gbig_d[i, :, :])
                gbig.append(t)
            w3s = wpool.tile([128, 24], F32R, name="w3s")
            nc.sync.dma_start(out=w3s[:, :], in_=w3s_d[:, :])
            u1 = wpool.tile([SROWS, 6], F32R, name="u1")
            nc.sync.dma_start(out=u1[:, :], in_=u1_d[:, :])
            bia = wpool.tile([128, 9], F32, name="bia")
            nc.sync.dma_start(out=bia[:, :], in_=bia_d[:, :])

            # --- persistent state tiles (per pair) ---
            stack, ytile, h2 = [], [], []
            for p in range(PAIRS):
                st = spool.tile([SROWS, PFREE], F32R, name=f"stack{p}")
                nc.sync.dma_start(out=st[:, :], in_=stk0_d[p, :, :])
                stack.append(st)
                yt = spool.tile([6, PFREE], F32, name=f"ytile{p}")
                nc.sync.dma_start(out=yt[:, :], in_=y0p_d[p, :, :])
                ytile.append(yt)
                h2.append([spool.tile([128, PFREE], F32R, name=f"h2_{p}_{i}")
                           for i in range(6)])

            # warm up the ACT tanh table outside the loop
            wu = wpool.tile([128, 1], F32R, name="wu")
            nc.scalar.activation(wu[:, :], bia[:, 8:9], TANH)

            W2B = gbig[5]      # block-diagonal W2
            G64 = gbig[6]      # h*A[6,4]*Ghat (stage 6's kt_4 substitute)
            S1B = sws[6]       # stage-1 stationary with folded update

            def uslice(u):
                return slice(u * FREE, (u + 1) * FREE)

            def mm_all(dst, wt, src_fn, start, stop):
                """One matmul per (pair, half) against a shared stationary."""
                for p in range(PAIRS):
                    for u in range(2):
                        nc.tensor.matmul(dst[p][:, uslice(u)], wt,
                                         src_fn(p)[:, uslice(u)],
                                         start=start, stop=stop,
                                         skip_group_check=True)

            def mm_pair(p, dst, wt, src, start, stop):
                for u in range(2):
                    nc.tensor.matmul(dst[:, uslice(u)], wt, src[:, uslice(u)],
                                     start=start, stop=stop,
                                     skip_group_check=True)

            def finish_update(pus):
                """hB6*W3 over h2_6, then exact fp32 y += delta, then refresh
                the stack's fp32r y image."""
                for p in range(PAIRS):
                    mm_pair(p, pus[p], w3s[:, 18:24], h2[p][5],
                            start=False, stop=True)
                for p in range(PAIRS):
                    nc.vector.tensor_add(out=ytile[p][:, :], in0=pus[p][:, :],
                                         in1=ytile[p][:, :])
                    nc.vector.tensor_copy(stack[p][0:6, :], ytile[p][:, :])

            def emit_substep(fold, pus_prev):
                """One Tsit5 substep.  fold=True folds the previous substep's
                update (pus_prev PSUM) into stage 1's stationaries, keeping
                the y add/copy off the critical path; returns this substep's
                update PSUM (finished here only if the caller won't fold)."""
                deferred = []          # kext mms+copies from previous stage
                pus = None
                for i in range(1, 7):
                    # --- layer-1 preactivation ---
                    ppre = [pbig.tile([128, PFREE], F32, name="ppre",
                                      tag=f"big{p}") for p in range(PAIRS)]
                    if i == 1 and fold:
                        # pre_1 = W1^T(y_old + sum hB_j k_j): S1B over stack
                        # + hB*Ghat over h2_4..6; finish pus_prev alongside.
                        # Stationary-major emission: one LDWEIGHTS per group.
                        mm_all(pus_prev, w3s[:, 18:24], lambda p: h2[p][5],
                               start=False, stop=True)
                        mm_all(ppre, S1B[:, :], lambda p: stack[p],
                               start=True, stop=False)
                        for j in range(3):
                            mm_all(ppre, gbig[7 + j], lambda p: h2[p][3 + j],
                                   start=False, stop=(j == 2))
                        for p in range(PAIRS):
                            nc.vector.tensor_add(out=ytile[p][:, :],
                                                 in0=pus_prev[p][:, :],
                                                 in1=ytile[p][:, :])
                            nc.vector.tensor_copy(stack[p][0:6, :],
                                                  ytile[p][:, :])
                    elif i == 6:
                        mm_all(ppre, sws[5][:, :], lambda p: stack[p],
                               start=True, stop=False)
                        mm_all(ppre, G64[:, :], lambda p: h2[p][3],
                               start=False, stop=False)
                        mm_all(ppre, gbig[4][:, :], lambda p: h2[p][4],
                               start=False, stop=True)
                    elif i == 1:
                        mm_all(ppre, sws[0][:, :], lambda p: stack[p],
                               start=True, stop=True)
                    else:
                        mm_all(ppre, sws[i - 1][:, :], lambda p: stack[p],
                               start=True, stop=False)
                        mm_all(ppre, gbig[i - 2][:, :], lambda p: h2[p][i - 2],
                               start=False, stop=True)
                    h1s = []
                    for p in range(PAIRS):
                        h1 = h1pool.tile([128, PFREE], F32R, name="h1",
                                         tag=f"h1{p}")
                        nc.scalar.activation(h1[:, :], ppre[p][:, :], TANH,
                                             bias=bia[:, i - 1:i] if not
                                             (i == 1 and fold) else bia[:, 7:8],
                                             scale=1.0)
                        h1s.append(h1)
                    # deferred kext from the previous stage fills PE idle here
                    for fn in deferred:
                        fn()
                    deferred = []
                    # --- layer 2 ---
                    pw2 = [pbig.tile([128, PFREE], F32, name="pw2",
                                     tag=f"big{p}") for p in range(PAIRS)]
                    mm_all(pw2, W2B[:, :], lambda p: h1s[p],
                           start=True, stop=True)
                    for p in range(PAIRS):
                        nc.scalar.activation(h2[p][i - 1][:, :], pw2[p][:, :],
                                             TANH, bias=bia[:, 6:7], scale=1.0)
                    if i == 6:
                        # update accumulation: U1 over the stack (kt_1..3)
                        # + hB4..5*W3 over h2_4, h2_5 (all ready here)
                        pus = [psmall.tile([6, PFREE], F32, name="pu",
                                           tag=f"small{p}")
                               for p in range(PAIRS)]
                        mm_all(pus, u1[:, :], lambda p: stack[p],
                               start=True, stop=False)
                        mm_all(pus, w3s[:, 6:12], lambda p: h2[p][3],
                               start=False, stop=False)
                        mm_all(pus, w3s[:, 12:18], lambda p: h2[p][4],
                               start=False, stop=False)
                    # --- ktilde extraction (j = 1..3), deferred one stage ---
                    if i <= 3:
                        def make_kext(i=i):
                            def emit():
                                pks = [psmall.tile([6, PFREE], F32, name="pk",
                                                   tag=f"small{p}")
                                       for p in range(PAIRS)]
                                mm_all(pks, w3s[:, 0:6], lambda p: h2[p][i - 1],
                                       start=True, stop=True)
                                r = _krow(i)
                                for p in range(PAIRS):
                                    nc.vector.tensor_copy(stack[p][r:r + 6, :],
                                                          pks[p][:, :])
                            return emit
                        deferred.append(make_kext())
                return pus

            with tc.For_i(0, n_intervals * loop_mult, 1,
                          hint_engines=(mybir.EngineType.PE,)) as iv:
                for _rep in range(body_reps):
                    pus = None
                    for sig in range(SUB):
                        pus = emit_substep(fold=(sig > 0), pus_prev=pus)
                    finish_update(pus)
                for p in range(PAIRS):
                    if static_save:
                        nc.sync.dma_start(out=ys_d[0, 6 * p:6 * p + 6, :],
                                          in_=ytile[p][:, :])
                    else:
                        nc.sync.dma_start(out=ys_d[ds(iv, 1), 6 * p:6 * p + 6, :],
                                          in_=ytile[p][:, :])

    nc.finalize()
    return nc


def build_timing_double(n_intervals: int = N_INT):
    """Timing-only variant: two interval bodies per save (wrong results)."""
    return build(n_intervals, body_reps=2)


_nc_cache = {}


def _get_nc(n_intervals: int):
    if n_intervals not in _nc_cache:
        _nc_cache[n_intervals] = build(n_intervals)
    return _nc_cache[n_intervals]


def prep_inputs(ts, y0, W1, b1, W2, b2, W3, b3):
    """Host-side precompute (float64) -> per-core input maps."""
    ts64 = np.asarray(ts, dtype=np.float64)
    h = (ts64[-1] - ts64[0]) / ((len(ts64) - 1) * SUB)
    W1_, b1_, W2_, b2_, W3_, b3_ = [np.asarray(a, dtype=np.float64)
                                    for a in (W1, b1, W2, b2, W3, b3)]
    y0_ = np.asarray(y0, dtype=np.float64)

    Ghat = W3_ @ W1_                    # [64, 64]
    g0 = b3_ @ W1_                      # [64]
    sumB = _B.sum()

    def pack2(v64):
        return np.concatenate([v64, v64])

    sws = np.zeros((7, SROWS, 128))
    for i in range(1, 7):
        s = sws[i - 1]
        s[0:3, 0:64] = W1_
        s[3:6, 64:128] = W1_
        for j in range(1, min(i - 1, 4)):   # j <= i-2 via stack (kt_1..3)
            r = _krow(j)
            s[r:r + 3, 0:64] = h * _A[i, j] * W1_
            s[r + 3:r + 6, 64:128] = h * _A[i, j] * W1_
    # S1B: stage-1 with the previous update folded in (y_old + sum hB_j k_j)
    s = sws[6]
    s[0:3, 0:64] = W1_
    s[3:6, 64:128] = W1_
    for j in range(1, 4):
        r = _krow(j)
        s[r:r + 3, 0:64] = h * _B[j] * W1_
        s[r + 3:r + 6, 64:128] = h * _B[j] * W1_

    gbig = np.zeros((10, 128, 128))
    for i in range(2, 7):               # h2_{i-1} direct contraction
        gbig[i - 2] = _blk(h * _A[i, i - 1] * Ghat)
    gbig[5] = _blk(W2_)
    gbig[6] = _blk(h * _A[6, 4] * Ghat)  # stage 6: kt_4 substitute
    for j in range(3):                   # S1B companions: hB4..6 * Ghat
        gbig[7 + j] = _blk(h * _B[4 + j] * Ghat)

    w3s = np.zeros((128, 24))
    w3s[0:64, 0:3] = W3_
    w3s[64:128, 3:6] = W3_
    for j in range(3):                  # hB4..hB6 W3-contractions (update)
        w3s[0:64, 6 * j + 6:6 * j + 9] = h * _B[4 + j] * W3_
        w3s[64:128, 6 * j + 9:6 * j + 12] = h * _B[4 + j] * W3_

    u1 = np.zeros((SROWS, 6))
    for j in range(1, 4):
        r = _krow(j)
        for d in range(3):
            u1[r + d, d] = h * _B[j]
            u1[r + 3 + d, 3 + d] = h * _B[j]
    u1[ONES_ROW, 0:3] = h * sumB * b3_
    u1[ONES_ROW, 3:6] = h * sumB * b3_

    bia = np.zeros((128, 9))
    for i in range(1, 7):
        bia[:, i - 1] = pack2(b1_ + h * _A[i, 1:i].sum() * g0)
    bia[:, 6] = pack2(b2_)
    bia[:, 7] = pack2(b1_ + h * sumB * g0)   # stage-1 bias with folded update

    sws = _round_fp32r(sws)
    gbig = _round_fp32r(gbig)
    w3s = _round_fp32r(w3s)
    u1 = _round_fp32r(u1)
    bia = bia.astype(np.float32)

    # y0 packing: batch index b = (2p+u)*1024 + hh*512 + n
    # pair tile rows hh*3+d, cols u*512+n
    y0r = y0_.astype(np.float32).reshape(N_CORES, PAIRS, 2, 2, FREE, D)
    y0p = y0r.transpose(0, 1, 3, 5, 2, 4).reshape(N_CORES, PAIRS, 6, PFREE)
    y0p = np.ascontiguousarray(y0p)

    stk0 = np.zeros((N_CORES, PAIRS, SROWS, PFREE), dtype=np.float32)
    stk0[:, :, 0:6, :] = _round_fp32r(y0p)
    stk0[:, :, ONES_ROW, :] = 1.0

    in_maps = []
    for c in range(N_CORES):
        in_maps.append({
            "stk0": np.ascontiguousarray(stk0[c]),
            "y0p": y0p[c],
            "sws": sws,
            "gbig": gbig,
            "w3s": w3s,
            "u1": u1,
            "biases": bia,
        })
    return in_maps


def assemble(results, y0, n_intervals: int = N_INT):
    """Per-core ys [n_int, 6*PAIRS, PFREE] -> full [n_int+1, B, 3]."""
    y0 = np.asarray(y0, dtype=np.float32)
    ys = np.empty((n_intervals + 1, B, 3), dtype=np.float32)
    ys[0] = y0
    shard = B // N_CORES
    for c in range(N_CORES):
        o = np.asarray(results[c]["ys"])
        # [t, p, hh, d, u, n] -> [t, p, u, hh, n, d]
        o = o.reshape(n_intervals, PAIRS, 2, 3, 2, FREE) \
             .transpose(0, 1, 4, 2, 5, 3).reshape(n_intervals, shard, 3)
        ys[1:, c * shard:(c + 1) * shard, :] = o
    return ys


def kernel(ts, y0, W1, b1, W2, b2, W3, b3):
    global LAST_EXEC_NS
    in_maps = prep_inputs(ts, y0, W1, b1, W2, b2, W3, b3)
    nc = _get_nc(N_INT)
    res = run_bass_kernel_spmd(nc, in_maps, list(range(N_CORES)))
    LAST_EXEC_NS = res.exec_time_ns
    return assemble(res.results, y0, N_INT)


if __name__ == "__main__":
    # smoke test with tiny interval count against a numpy reference
    rng = np.random.default_rng(0)
    ts = np.linspace(0, 1, T, dtype=np.float32)
    y0 = rng.standard_normal((B, D)).astype(np.float32)
    W1 = (rng.standard_normal((D, W)) / np.sqrt(D)).astype(np.float32)
    W2 = (rng.standard_normal((W, W)) / np.sqrt(W)).astype(np.float32)
    W3 = (rng.standard_normal((W, D)) / np.sqrt(W)).astype(np.float32)
    b1 = np.zeros(W, np.float32)
    b2 = np.zeros(W, np.float32)
    b3 = np.zeros(D, np.float32)

    n_int = 2
    in_maps = prep_inputs(ts, y0, W1, b1, W2, b2, W3, b3)
    nc = build(n_int)
    res = run_bass_kernel_spmd(nc, in_maps, list(range(N_CORES)))
    ys = assemble(res.results, y0, n_int)

    # numpy reference (float64) for the first n_int*SUB steps
    def vf(y):
        h1 = np.tanh(y @ W1.astype(np.float64) + b1)
        hh = np.tanh(h1 @ W2.astype(np.float64) + b2)
        return hh @ W3.astype(np.float64) + b3

    h = float(ts[1] - ts[0]) / SUB
    y = y0.astype(np.float64)
    outs = [y0.astype(np.float64)]
    for t in range(n_int * SUB):
        k1 = vf(y)
        k2 = vf(y + h * (_A[2, 1] * k1))
        k3 = vf(y + h * (_A[3, 1] * k1 + _A[3, 2] * k2))
        k4 = vf(y + h * (_A[4, 1] * k1 + _A[4, 2] * k2 + _A[4, 3] * k3))
        k5 = vf(y + h * (_A[5, 1] * k1 + _A[5, 2] * k2 + _A[5, 3] * k3 + _A[5, 4] * k4))
        k6 = vf(y + h * (_A[6, 1] * k1 + _A[6, 2] * k2 + _A[6, 3] * k3
                         + _A[6, 4] * k4 + _A[6, 5] * k5))
        y = y + h * (_B[1] * k1 + _B[2] * k2 + _B[3] * k3 + _B[4] * k4
                     + _B[5] * k5 + _B[6] * k6)
        if (t + 1) % SUB == 0:
            outs.append(y.copy())
    ref = np.stack(outs)
    err = np.abs(ys - ref).max()
    scale = np.abs(ref).max()
    print(f"smoke n_int={n_int}: maxabs={err:.3e} rel={err/scale:.3e} scale={scale:.3f}")


# revision 45
# speedup vs baseline: 1.1359x; 1.1359x over previous
"""Trainium2 Bass kernel for nn_NeuralODE: 196 Tsit5 steps of a 3->64->64->3
tanh MLP vector field over batch 32768, data-parallel across 8 NeuronCores.

Formulation:
  State is y itself ([3] per batch row), kept exactly in fp32 in a [6, 1024]
  tile per wave-pair (2 batch halves x 3 dims on partitions).  A packed fp32r
  "stack" tile [102, 1024] per pair holds an fp32r image of y (rows 0-5), a
  ones row (6), and kt_j at 32-aligned bases 32/64/96 for j=1..3 (engine
  writes must be 32-partition aligned), where kt_j := h2_j @ W3.  Per stage i
  the layer-1 preactivation
     pre_i = W1^T y + sum_{j<i} (h A_ij) W1^T k_j + const
  is ONE K=102 matmul over the stack (y and kt_{j<=i-2}) plus ONE
  accumulating K=128 matmul of h2_{i-1} against scaled Ghat = W3 @ W1 (stage
  6 adds one more for h2_4, whose kt has no aligned slot).  W2 is a
  block-diagonal K=128 matmul on [64 feats x 2 halves].  The step update
     y' = y + sum_j (h B_j) k_j
  is 4 accumulating matmuls into a [6, 1024] PSUM (stack kt_1..3 +
  W3-contractions of h2_4..6), an exact fp32 VectorE add into the y tile,
  and a VectorE copy refreshing the stack's fp32r y image.  Constants
  (b1/b2/b3 terms) fold into ACT bias columns / stationary rows (zero here).

  Compute granularity: 4 PE waves of 512 columns (25 matmul passes per step
  per wave), PAIRED into 2 streams for ScalarE/VectorE so each tanh covers
  [128, 1024] (halves the per-instruction overhead on the binding engine).
  kext matmuls+copies are emitted one stage late so they fill PE idle slots
  instead of blocking the next stage's preacts.  PSUM: 2x [128,1024] big
  tags + 2x [6,1024] small tags = exactly 8 banks.
"""
import numpy as np

import concourse.bacc as bacc
import concourse.bass as bass
import concourse.mybir as mybir
from concourse.bass import ds
from concourse.tile import TileContext
from concourse.bass_utils import run_bass_kernel_spmd

F32 = mybir.dt.float32
F32R = mybir.dt.float32r
TANH = mybir.ActivationFunctionType.Tanh

N_CORES = 8
T, B, D, W = 50, 32768, 3, 64
SUB = 4
N_INT = T - 1            # 49 save intervals
WAVES = 4                # PE streams of FREE columns each
PAIRS = WAVES // 2       # ACT/DVE streams of 2*FREE columns each
FREE = B // N_CORES // WAVES // 2   # 512
PFREE = 2 * FREE         # 1024: free dim of paired tiles
# NOTE: each matmul must write a full half of a paired PSUM tile (one bank);
# two accumulation groups must never share a PSUM bank (start=True clears
# has_written for the whole bank).
SROWS = 102              # stack rows: 6 y, 1 ones, kt_j at 32j..32j+5
ONES_ROW = 6

# Tsit5 tableau (matches reference.py)
_A = np.zeros((7, 7))
_A[2, 1] = 0.161
_A[3, 1], _A[3, 2] = -0.008480655492356989, 0.335480655492357
_A[4, 1], _A[4, 2], _A[4, 3] = 2.8971530571054935, -6.359448489975075, 4.3622954328695815
_A[5, 1], _A[5, 2], _A[5, 3], _A[5, 4] = (
    5.325864828439257, -11.748883564062828, 7.4955393428898365, -0.09249506636175525)
_A[6, 1], _A[6, 2], _A[6, 3], _A[6, 4], _A[6, 5] = (
    5.86145544294642, -12.92096931784711, 8.159367898576159,
    -0.071584973281401, -0.028269050394068383)
_B = np.array([0.0, 0.09646076681806523, 0.01, 0.4798896504144996,
               1.379008574103742, -3.290069515436081, 2.324710524099774])

LAST_EXEC_NS = None


def _krow(j: int) -> int:
    """First stack row of ktilde_j (j = 1..3) — 32-partition aligned."""
    return 32 * j


def _round_fp32r(x: np.ndarray) -> np.ndarray:
    """Round fp32 array to the fp32r grid (11-bit mantissa, RNE-ish)."""
    u = np.ascontiguousarray(np.asarray(x, dtype=np.float32)).view(np.uint32)
    r = (u + np.uint32(0x7FF) + ((u >> np.uint32(12)) & np.uint32(1))) & np.uint32(0xFFFFF000)
    return r.view(np.float32)


def _blk(m64: np.ndarray) -> np.ndarray:
    """Duplicate a [64,64] matrix into a block-diagonal [128,128]."""
    z = np.zeros((128, 128), dtype=np.float64)
    z[0:64, 0:64] = m64
    z[64:128, 64:128] = m64
    return z


def build(n_intervals: int = N_INT, body_reps: int = 1, loop_mult: int = 1,
          static_save: bool = False):
    nc = bacc.Bacc(None, target_bir_lowering=False)

    stk0_d = nc.dram_tensor("stk0", [PAIRS, SROWS, PFREE], F32R, kind="ExternalInput")
    y0p_d = nc.dram_tensor("y0p", [PAIRS, 6, PFREE], F32, kind="ExternalInput")
    sws_d = nc.dram_tensor("sws", [7, SROWS, 128], F32R, kind="ExternalInput")
    gbig_d = nc.dram_tensor("gbig", [10, 128, 128], F32R, kind="ExternalInput")
    w3s_d = nc.dram_tensor("w3s", [128, 24], F32R, kind="ExternalInput")
    u1_d = nc.dram_tensor("u1", [SROWS, 6], F32R, kind="ExternalInput")
    bia_d = nc.dram_tensor("biases", [128, 9], F32, kind="ExternalInput")
    group = 7 if n_intervals % 7 == 0 else 1
    niter = n_intervals // group
    ys_d = nc.dram_tensor("ys", [niter, group * 6 * PAIRS, PFREE], F32,
                          kind="ExternalOutput")

    with TileContext(nc) as tc:
        with tc.tile_pool(name="wpool", bufs=1) as wpool, \
             tc.tile_pool(name="spool", bufs=1) as spool, \
             tc.tile_pool(name="h1pool", bufs=2) as h1pool, \
             tc.tile_pool(name="pbig", bufs=1, space="PSUM") as pbig, \
             tc.tile_pool(name="psmall", bufs=1, space="PSUM") as psmall:

            # --- weights / constants to SBUF ---
            sws = []
            for i in range(7):
                t = wpool.tile([SROWS, 128], F32R, name=f"sw{i}")
                nc.sync.dma_start(out=t[:, :], in_=sws_d[i, :, :])
                sws.append(t)
            gbig = []
            for i in range(10):
                t = wpool.tile([128, 128], F32R, name=f"gb{i}")
                nc.sync.dma_start(out=t[:, :], in_=gbig_d[i, :, :])
                gbig.append(t)
            w3s = wpool.tile([128, 24], F32R, name="w3s")
            nc.sync.dma_start(out=w3s[:, :], in_=w3s_d[:, :])
            u1 = wpool.tile([SROWS, 6], F32R, name="u1")
            nc.sync.dma_start(out=u1[:, :], in_=u1_d[:, :])
            bia = wpool.tile([128, 9], F32, name="bia")
            nc.sync.dma_start(out=bia[:, :], in_=bia_d[:, :])

            # --- persistent state tiles (per pair) ---
            stack, ytile, ysave, h2 = [], [], [], []
            for p in range(PAIRS):
                st = spool.tile([SROWS, PFREE], F32R, name=f"stack{p}")
                nc.sync.dma_start(out=st[:, :], in_=stk0_d[p, :, :])
                stack.append(st)
                yt = spool.tile([6, PFREE], F32, name=f"ytile{p}")
                nc.sync.dma_start(out=yt[:, :], in_=y0p_d[p, :, :])
                ytile.append(yt)
                ysave.append(spool.tile([6, PFREE], F32, name=f"ysave{p}"))
                h2.append([spool.tile([128, PFREE], F32R, name=f"h2_{p}_{i}")
                           for i in range(6)])

            # warm up the ACT tanh table outside the loop
            wu = wpool.tile([128, 1], F32R, name="wu")
            nc.scalar.activation(wu[:, :], bia[:, 8:9], TANH)

            W2B = gbig[5]      # block-diagonal W2
            G64 = gbig[6]      # h*A[6,4]*Ghat (stage 6's kt_4 substitute)
            S1B = sws[6]       # stage-1 stationary with folded update

            def uslice(u):
                return slice(u * FREE, (u + 1) * FREE)

            def mm_all(dst, wt, src_fn, start, stop):
                """One matmul per (pair, half) against a shared stationary."""
                for p in range(PAIRS):
                    for u in range(2):
                        nc.tensor.matmul(dst[p][:, uslice(u)], wt,
                                         src_fn(p)[:, uslice(u)],
                                         start=start, stop=stop,
                                         skip_group_check=True)

            def mm_pair(p, dst, wt, src, start, stop):
                for u in range(2):
                    nc.tensor.matmul(dst[:, uslice(u)], wt, src[:, uslice(u)],
                                     start=start, stop=stop,
                                     skip_group_check=True)

            def finish_interval(pus, last):
                """hB6*W3 over h2_6 completes the update.  last=True:
                commit y (exact fp32 add + stack f32r image refresh) for the
                next For_i iteration.  last=False: only materialize y_next
                into ysave for the save DMA; the commit happens in the next
                interval's folded stage 1."""
                for p in range(PAIRS):
                    mm_pair(p, pus[p], w3s[:, 18:24], h2[p][5],
                            start=False, stop=True)
                for p in range(PAIRS):
                    if last:
                        nc.vector.tensor_add(out=ytile[p][:, :],
                                             in0=pus[p][:, :],
                                             in1=ytile[p][:, :])
                        nc.vector.tensor_copy(stack[p][0:6, :],
                                              ytile[p][:, :])
                    else:
                        nc.vector.tensor_add(out=ysave[p][:, :],
                                             in0=pus[p][:, :],
                                             in1=ytile[p][:, :])

            def emit_substep(fold, pus_prev, w36_prev=True):
                """One Tsit5 substep.  fold=True folds the previous substep's
                update (pus_prev PSUM) into stage 1's stationaries, keeping
                the y add/copy off the critical path; returns this substep's
                update PSUM.  w36_prev=False: pus_prev is already complete
                (finish_interval emitted its last matmul)."""
                deferred = []          # kext mms+copies from previous stage
                pus = None
                for i in range(1, 7):
                    # --- layer-1 preactivation ---
                    ppre = [pbig.tile([128, PFREE], F32, name="ppre",
                                      tag=f"big{p}") for p in range(PAIRS)]
                    if i == 1 and fold:
                        # pre_1 = W1^T(y_old + sum hB_j k_j): S1B over stack
                        # + hB*Ghat over h2_4..6; finish pus_prev alongside.
                        # Stationary-major emission: one LDWEIGHTS per group.
                        if w36_prev:
                            mm_all(pus_prev, w3s[:, 18:24],
                                   lambda p: h2[p][5],
                                   start=False, stop=True)
                        mm_all(ppre, S1B[:, :], lambda p: stack[p],
                               start=True, stop=False)
                        for j in range(3):
                            mm_all(ppre, gbig[7 + j], lambda p: h2[p][3 + j],
                                   start=False, stop=(j == 2))
                        for p in range(PAIRS):
                            nc.vector.tensor_add(out=ytile[p][:, :],
                                                 in0=pus_prev[p][:, :],
                                                 in1=ytile[p][:, :])
                            nc.vector.tensor_copy(stack[p][0:6, :],
                                                  ytile[p][:, :])
                    elif i == 6:
                        mm_all(ppre, sws[5][:, :], lambda p: stack[p],
                               start=True, stop=False)
                        mm_all(ppre, G64[:, :], lambda p: h2[p][3],
                               start=False, stop=False)
                        mm_all(ppre, gbig[4][:, :], lambda p: h2[p][4],
                               start=False, stop=True)
                    elif i == 1:
                        mm_all(ppre, sws[0][:, :], lambda p: stack[p],
                               start=True, stop=True)
                    else:
                        mm_all(ppre, sws[i - 1][:, :], lambda p: stack[p],
                               start=True, stop=False)
                        mm_all(ppre, gbig[i - 2][:, :], lambda p: h2[p][i - 2],
                               start=False, stop=True)
                    h1s = []
                    for p in range(PAIRS):
                        h1 = h1pool.tile([128, PFREE], F32R, name="h1",
                                         tag=f"h1{p}")
                        nc.scalar.activation(h1[:, :], ppre[p][:, :], TANH,
                                             bias=bia[:, i - 1:i] if not
                                             (i == 1 and fold) else bia[:, 7:8],
                                             scale=1.0)
                        h1s.append(h1)
                    # deferred kext from the previous stage fills PE idle here
                    for fn in deferred:
                        fn()
                    deferred = []
                    # --- layer 2 ---
                    pw2 = [pbig.tile([128, PFREE], F32, name="pw2",
                                     tag=f"big{p}") for p in range(PAIRS)]
                    mm_all(pw2, W2B[:, :], lambda p: h1s[p],
                           start=True, stop=True)
                    for p in range(PAIRS):
                        nc.scalar.activation(h2[p][i - 1][:, :], pw2[p][:, :],
                                             TANH, bias=bia[:, 6:7], scale=1.0)
                    if i == 6:
                        # update accumulation: U1 over the stack (kt_1..3)
                        # + hB4..5*W3 over h2_4, h2_5 (all ready here)
                        pus = [psmall.tile([6, PFREE], F32, name="pu",
                                           tag=f"small{p}")
                               for p in range(PAIRS)]
                        mm_all(pus, u1[:, :], lambda p: stack[p],
                               start=True, stop=False)
                        mm_all(pus, w3s[:, 6:12], lambda p: h2[p][3],
                               start=False, stop=False)
                        mm_all(pus, w3s[:, 12:18], lambda p: h2[p][4],
                               start=False, stop=False)
                    # --- ktilde extraction (j = 1..3), deferred one stage ---
                    if i <= 3:
                        def make_kext(i=i):
                            def emit():
                                pks = [psmall.tile([6, PFREE], F32, name="pk",
                                                   tag=f"small{p}")
                                       for p in range(PAIRS)]
                                mm_all(pks, w3s[:, 0:6], lambda p: h2[p][i - 1],
                                       start=True, stop=True)
                                r = _krow(i)
                                for p in range(PAIRS):
                                    nc.vector.tensor_copy(stack[p][r:r + 6, :],
                                                          pks[p][:, :])
                            return emit
                        deferred.append(make_kext())
                return pus

            with tc.For_i(0, niter * loop_mult, 1,
                          hint_engines=(mybir.EngineType.PE,)) as iv:
                for rep in range(body_reps):
                    pus = None
                    for g in range(group):
                        for sig in range(SUB):
                            first = (rep == 0 and g == 0 and sig == 0)
                            pus = emit_substep(fold=not first, pus_prev=pus,
                                               w36_prev=(sig > 0))
                        last = (g == group - 1)
                        finish_interval(pus, last)
                        src_t = ytile if last else ysave
                        for p in range(PAIRS):
                            row = (g * PAIRS + p) * 6
                            if static_save:
                                nc.sync.dma_start(
                                    out=ys_d[0, row:row + 6, :],
                                    in_=src_t[p][:, :])
                            else:
                                nc.sync.dma_start(
                                    out=ys_d[ds(iv, 1), row:row + 6, :],
                                    in_=src_t[p][:, :])

    nc.finalize()
    return nc


def build_timing_double(n_intervals: int = N_INT):
    """Timing-only variant: two interval bodies per save (wrong results)."""
    return build(n_intervals, body_reps=2)


_nc_cache = {}


def _get_nc(n_intervals: int):
    if n_intervals not in _nc_cache:
        _nc_cache[n_intervals] = build(n_intervals)
    return _nc_cache[n_intervals]


def prep_inputs(ts, y0, W1, b1, W2, b2, W3, b3):
    """Host-side precompute (float64) -> per-core input maps."""
    ts64 = np.asarray(ts, dtype=np.float64)
    h = (ts64[-1] - ts64[0]) / ((len(ts64) - 1) * SUB)
    W1_, b1_, W2_, b2_, W3_, b3_ = [np.asarray(a, dtype=np.float64)
                                    for a in (W1, b1, W2, b2, W3, b3)]
    y0_ = np.asarray(y0, dtype=np.float64)

    Ghat = W3_ @ W1_                    # [64, 64]
    g0 = b3_ @ W1_                      # [64]
    sumB = _B.sum()

    def pack2(v64):
        return np.concatenate([v64, v64])

    sws = np.zeros((7, SROWS, 128))
    for i in range(1, 7):
        s = sws[i - 1]
        s[0:3, 0:64] = W1_
        s[3:6, 64:128] = W1_
        for j in range(1, min(i - 1, 4)):   # j <= i-2 via stack (kt_1..3)
            r = _krow(j)
            s[r:r + 3, 0:64] = h * _A[i, j] * W1_
            s[r + 3:r + 6, 64:128] = h * _A[i, j] * W1_
    # S1B: stage-1 with the previous update folded in (y_old + sum hB_j k_j)
    s = sws[6]
    s[0:3, 0:64] = W1_
    s[3:6, 64:128] = W1_
    for j in range(1, 4):
        r = _krow(j)
        s[r:r + 3, 0:64] = h * _B[j] * W1_
        s[r + 3:r + 6, 64:128] = h * _B[j] * W1_

    gbig = np.zeros((10, 128, 128))
    for i in range(2, 7):               # h2_{i-1} direct contraction
        gbig[i - 2] = _blk(h * _A[i, i - 1] * Ghat)
    gbig[5] = _blk(W2_)
    gbig[6] = _blk(h * _A[6, 4] * Ghat)  # stage 6: kt_4 substitute
    for j in range(3):                   # S1B companions: hB4..6 * Ghat
        gbig[7 + j] = _blk(h * _B[4 + j] * Ghat)

    w3s = np.zeros((128, 24))
    w3s[0:64, 0:3] = W3_
    w3s[64:128, 3:6] = W3_
    for j in range(3):                  # hB4..hB6 W3-contractions (update)
        w3s[0:64, 6 * j + 6:6 * j + 9] = h * _B[4 + j] * W3_
        w3s[64:128, 6 * j + 9:6 * j + 12] = h * _B[4 + j] * W3_

    u1 = np.zeros((SROWS, 6))
    for j in range(1, 4):
        r = _krow(j)
        for d in range(3):
            u1[r + d, d] = h * _B[j]
            u1[r + 3 + d, 3 + d] = h * _B[j]
    u1[ONES_ROW, 0:3] = h * sumB * b3_
    u1[ONES_ROW, 3:6] = h * sumB * b3_

    bia = np.zeros((128, 9))
    for i in range(1, 7):
        bia[:, i - 1] = pack2(b1_ + h * _A[i, 1:i].sum() * g0)
    bia[:, 6] = pack2(b2_)
    bia[:, 7] = pack2(b1_ + h * sumB * g0)   # stage-1 bias with folded update

    sws = _round_fp32r(sws)
    gbig = _round_fp32r(gbig)
    w3s = _round_fp32r(w3s)
    u1 = _round_fp32r(u1)
    bia = bia.astype(np.float32)

    # y0 packing: batch index b = (2p+u)*1024 + hh*512 + n
    # pair tile rows hh*3+d, cols u*512+n
    y0r = y0_.astype(np.float32).reshape(N_CORES, PAIRS, 2, 2, FREE, D)
    y0p = y0r.transpose(0, 1, 3, 5, 2, 4).reshape(N_CORES, PAIRS, 6, PFREE)
    y0p = np.ascontiguousarray(y0p)

    stk0 = np.zeros((N_CORES, PAIRS, SROWS, PFREE), dtype=np.float32)
    stk0[:, :, 0:6, :] = _round_fp32r(y0p)
    stk0[:, :, ONES_ROW, :] = 1.0

    in_maps = []
    for c in range(N_CORES):
        in_maps.append({
            "stk0": np.ascontiguousarray(stk0[c]),
            "y0p": y0p[c],
            "sws": sws,
            "gbig": gbig,
            "w3s": w3s,
            "u1": u1,
            "biases": bia,
        })
    return in_maps


def assemble(results, y0, n_intervals: int = N_INT):
    """Per-core ys [n_int, 6*PAIRS, PFREE] -> full [n_int+1, B, 3]."""
    y0 = np.asarray(y0, dtype=np.float32)
    ys = np.empty((n_intervals + 1, B, 3), dtype=np.float32)
    ys[0] = y0
    shard = B // N_CORES
    for c in range(N_CORES):
        o = np.asarray(results[c]["ys"])
        # [t, p, hh, d, u, n] -> [t, p, u, hh, n, d]
        o = o.reshape(n_intervals, PAIRS, 2, 3, 2, FREE) \
             .transpose(0, 1, 4, 2, 5, 3).reshape(n_intervals, shard, 3)
        ys[1:, c * shard:(c + 1) * shard, :] = o
    return ys


def kernel(ts, y0, W1, b1, W2, b2, W3, b3):
    global LAST_EXEC_NS
    in_maps = prep_inputs(ts, y0, W1, b1, W2, b2, W3, b3)
    nc = _get_nc(N_INT)
    res = run_bass_kernel_spmd(nc, in_maps, list(range(N_CORES)))
    LAST_EXEC_NS = res.exec_time_ns
    return assemble(res.results, y0, N_INT)


if __name__ == "__main__":
    # smoke test with tiny interval count against a numpy reference
    rng = np.random.default_rng(0)
    ts = np.linspace(0, 1, T, dtype=np.float32)
    y0 = rng.standard_normal((B, D)).astype(np.float32)
    W1 = (rng.standard_normal((D, W)) / np.sqrt(D)).astype(np.float32)
    W2 = (rng.standard_normal((W, W)) / np.sqrt(W)).astype(np.float32)
    W3 = (rng.standard_normal((W, D)) / np.sqrt(W)).astype(np.float32)
    b1 = np.zeros(W, np.float32)
    b2 = np.zeros(W, np.float32)
    b3 = np.zeros(D, np.float32)

    n_int = 2
    in_maps = prep_inputs(ts, y0, W1, b1, W2, b2, W3, b3)
    nc = build(n_int)
    res = run_bass_kernel_spmd(nc, in_maps, list(range(N_CORES)))
    ys = assemble(res.results, y0, n_int)

    # numpy reference (float64) for the first n_int*SUB steps
    def vf(y):
        h1 = np.tanh(y @ W1.astype(np.float64) + b1)
        hh = np.tanh(h1 @ W2.astype(np.float64) + b2)
        return hh @ W3.astype(np.float64) + b3

    h = float(ts[1] - ts[0]) / SUB
    y = y0.astype(np.float64)
    outs = [y0.astype(np.float64)]
    for t in range(n_int * SUB):
        k1 = vf(y)
        k2 = vf(y + h * (_A[2, 1] * k1))
        k3 = vf(y + h * (_A[3, 1] * k1 + _A[3, 2] * k2))
        k4 = vf(y + h * (_A[4, 1] * k1 + _A[4, 2] * k2 + _A[4, 3] * k3))
        k5 = vf(y + h * (_A[5, 1] * k1 + _A[5, 2] * k2 + _A[5, 3] * k3 + _A[5, 4] * k4))
        k6 = vf(y + h * (_A[6, 1] * k1 + _A[6, 2] * k2 + _A[6, 3] * k3
                         + _A[6, 4] * k4 + _A[6, 5] * k5))
        y = y + h * (_B[1] * k1 + _B[2] * k2 + _B[3] * k3 + _B[4] * k4
                     + _B[5] * k5 + _B[6] * k6)
        if (t + 1) % SUB == 0:
            outs.append(y.copy())
    ref = np.stack(outs)
    err = np.abs(ys - ref).max()
    scale = np.abs(ref).max()
    print(f"smoke n_int={n_int}: maxabs={err:.3e} rel={err/scale:.3e} scale={scale:.3f}")
